# revision 29
# baseline (speedup 1.0000x reference)
"""Trainium2 Bass kernel for CRF mean log-likelihood (B=128, S=512, T=256).

Strategy: data-parallel over batch (16 sequences per core, 8 cores). The
forward-algorithm log-partition is computed in exponential space so the
per-step T x T logsumexp becomes a PE matmul:

    alpha_s = (E^T alpha_{s-1}) * exp(emit_s - delta)     E = exp(trans)

with a constant per-step shift delta ~= log(T) + 1/2 (keeps the state in a
narrow dynamic range; validated drift < +-6 in log space).

The chain is latency-bound (matmul -> elementwise multiply -> matmul), so
the sequence is processed FROM BOTH ENDS simultaneously (meet in the
middle):
  forward:  alpha_s = (E^T alpha_{s-1}) . ee_s          s = 1..Rf
  backward: u_s = (E u_{s+1}) . ee_s   (u_s=gamma_s.ee_s), s = S-2..Rf+1
  Z        = (E^T alpha_Rf)^T  u_{Rf+1}

Per-round critical cycle = mm(172ns fixed SBUF latency) + sem + mult
(175ns, PSUM access) + sem + second-mult serialization.  To cut the last
term the two block-halves' multiplies run on DIFFERENT engines: block 0 on
DVE (vector), block 1 on Pool (gpsimd).  All emission-chunk DMAs issue
from Sync so Pool stays clean; init DMAs are spread across engines and
combined (tr+trt one param, st+en one param) to shorten the pre-chain
head.

The gold (numerator) score is O(B*S) gather work — computed on host.
"""
import numpy as np

B, S, T = 128, 512, 256
NCORES = 8
BPC = B // NCORES          # batch per core = 16
G = 1                      # batch groups per core (chains = 2*G)
GB = BPC // G
W = 128                    # steps per emissions chunk
FIRST = 8                  # steps in the init-critical first piece
BULK = 32                  # steps per bulk DMA/exp piece
DELTA = 6.045              # per-step log-space shift ~ log(256) + 0.5
KEEP_MM_WAITS = True       # skip bacc's move_matmul_waits_to_ldweights

_cache = {}


def _pieces(n, descending, first_small):
    """Piece boundaries for chunk streaming, in consumption order.  First
    chunks ramp FIRST -> BULK -> 2*BULK; steady-state chunks use 2*BULK
    pieces (fewer piece boundaries = fewer DVE sem-guard bubbles)."""
    widths, rem = [], n
    for w in ([FIRST, BULK] if first_small else []):
        w = min(w, rem)
        if w:
            widths.append(w)
            rem -= w
    while rem > 0:
        w = min(2 * BULK, rem)
        widths.append(w)
        rem -= w
    pieces, pos = [], 0
    for w in widths:
        pieces.append((pos, pos + w))
        pos += w
    if descending:
        pieces = [(n - b, n - a) for a, b in pieces]
    return pieces


def build_nc(n_steps=S):
    import concourse.bass as bass
    import concourse.tile as tile
    from concourse import bacc, mybir
    from contextlib import ExitStack

    f32 = mybir.dt.float32
    bf16 = mybir.dt.bfloat16
    Exp = mybir.ActivationFunctionType.Exp

    assert n_steps >= 4
    Rf = (n_steps - 2) // 2          # forward DVE-rounds (alpha_1..alpha_Rf)

    nc = bacc.Bacc()
    em = nc.declare_dram_parameter("em", [2, 128, n_steps, BPC], bf16,
                                   isOutput=False)
    # trx[i] = [tr[i] blocks j=0,1 | trt[i] blocks j=0,1] along slot dim
    trx = nc.declare_dram_parameter("trx", [2, 128, 4, 128], bf16,
                                    isOutput=False)
    # sew[p, i, c]: c=0 start, c=1 end; i = contraction half
    sew = nc.declare_dram_parameter("sew", [128, 2, 2], f32, isOutput=False)
    out = nc.declare_dram_parameter("out", [1, BPC], f32, isOutput=True)

    with ExitStack() as ctx:
        tc = ctx.enter_context(tile.TileContext(nc))
        const = ctx.enter_context(tc.tile_pool(name="const", bufs=1))
        emf = ctx.enter_context(tc.tile_pool(name="emf", bufs=3))
        eef = ctx.enter_context(tc.tile_pool(name="eef", bufs=3))
        emb = ctx.enter_context(tc.tile_pool(name="emb", bufs=3))
        eeb = ctx.enter_context(tc.tile_pool(name="eeb", bufs=3))
        ppool = ctx.enter_context(tc.tile_pool(name="p", bufs=4))
        rpool = ctx.enter_context(tc.tile_pool(name="rn", bufs=2))
        qpool = ctx.enter_context(tc.tile_pool(name="q", bufs=1, space="PSUM"))
        spool = ctx.enter_context(tc.tile_pool(name="s", bufs=2, space="PSUM"))

        # ---- one-time constants ----
        # Weight staging: one DMA + one big exp ACT per contraction half i.
        # Eall[i][:, 0:2, :] = E blocks (fwd lhsT), [:, 2:4, :] = Et (bwd).
        # trx already holds exp(transitions) (computed on host), so the
        # weight tiles are DMA'd directly — no device-side exp ACTs on the
        # first-matmul critical path.
        Eall = [const.tile([128, 4, 128], bf16, tag=f"Eall{i}", name=f"Eall{i}")
                for i in range(2)]
        for i in range(2):
            nc.sync.dma_start(out=Eall[i], in_=trx[i])
        E = [[Eall[i][:, j, :] for j in range(2)] for i in range(2)]
        Et = [[Eall[i][:, 2 + j, :] for j in range(2)] for i in range(2)]

        sews = rpool.tile([128, 2, 2], f32, tag="sewstage", name="sewstage")
        nc.gpsimd.dma_start(out=sews, in_=sew[:, :, :])
        # PE p-state warm-up: the tensor engine clock ramps with sustained
        # use (full speed after ~3us).  The PE is otherwise idle until the
        # first real matmul (~11us), so early rounds would pay ~2x-slow
        # weight loads.  Dummy matmuls (alternating weights to force real
        # ldweights) keep it executing through the head.
        warmW = []
        for k in range(2):
            wt = const.tile([128, 128], bf16, tag=f"warmW{k}", name=f"warmW{k}")
            nc.vector.memset(wt, 0.0)
            warmW.append(wt)
        warmS = const.tile([128, GB], bf16, tag="warmS", name="warmS")
        nc.vector.memset(warmS, 0.0)
        wq = spool.tile([128, GB], f32, tag="warmq", name="warmq")
        NWARM = 40
        for k in range(NWARM):
            nc.tensor.matmul(wq, warmW[k % 2], warmS,
                             start=(k == 0), stop=(k == NWARM - 1))
        onesf = const.tile([128, 1], f32, tag="onesf", name="onesf")
        nc.vector.memset(onesf, 1.0)
        dbias = const.tile([128, 1], f32, tag="dbias", name="dbias")
        nc.vector.memset(dbias, -DELTA)
        st_t = [sews[:, i, 0:1] for i in range(2)]
        ben = []
        for i in range(2):
            t = const.tile([128, 1], f32, tag=f"ben{i}", name=f"ben{i}")
            nc.vector.tensor_add(t, sews[:, i, 1:2], dbias)  # end - delta
            ben.append(t)

        # ---- emissions chunk streaming (per direction) ----
        # Stream each chunk in pieces (DMA pair + exp ACT per piece), ordered
        # by consumption direction; all chunk DMAs issue from Sync so the
        # Pool engine stays free for the per-round multiplies.
        def load_chunk(c, pool, eepool_, nm, descending=False,
                       mode="full", tiles=None, dma_eng=None, piece_idx=None):
            """mode: full (fresh chunk, dma+exp) | first_dma (alloc, dma of
            the small first piece only) | first_exp (exp of that piece) |
            rest (dma+exp of remaining pieces; piece_idx selects one)."""
            s0, s1 = c * W, min(n_steps, (c + 1) * W)
            n = s1 - s0
            if tiles is None:
                t = pool.tile([128, 2, W, BPC], bf16, tag="emchunk",
                              name=f"em{nm}")
                te = eepool_.tile([128, 2, W, BPC], bf16, tag="eechunk",
                                  name=f"ee{nm}")
            else:
                t, te = tiles
            pieces = _pieces(n, descending, mode != "full")
            if mode in ("first_dma", "first_exp"):
                pieces = pieces[:1]
            elif mode == "rest":
                pieces = pieces[1:]
                if piece_idx is not None:
                    pieces = pieces[piece_idx:piece_idx + 1]
            for a, b in pieces:
                if mode != "first_exp":
                    for i in range(2):
                        (dma_eng or nc.gpsimd).dma_start(
                            out=t[:, i, a:b, :],
                            in_=em[i, :, s0 + a:s0 + b, :])
                if mode != "first_dma":
                    nc.scalar.activation(te[:, :, a:b, :], t[:, :, a:b, :],
                                         Exp, bias=dbias)
            return t, te

        # ---- chain state ----
        # First the two init-critical pieces + the state inits, then the bulk
        # of both chunks — keeps the first matmul off the DMA/ACT queues.
        cf = 0                       # forward chunk index
        cb = (n_steps - 1) // W      # backward chunk index
        tf = load_chunk(cf, emf, eef, "f0", mode="first_dma",
                        dma_eng=nc.gpsimd)
        same = (cb == cf)
        tb = tf if same else load_chunk(cb, emb, eeb, "b0", descending=True,
                                        mode="first_dma", dma_eng=nc.sync)
        em_f, ee_f = tf
        em_b, ee_b = tb

        p = []   # forward states per group
        u = []   # backward states per group
        pts = [ppool.tile([128, 2, GB], bf16, tag=f"pf{g}", name=f"pf{g}")
               for g in range(G)]
        uts = [ppool.tile([128, 2, GB], bf16, tag=f"pb{g}", name=f"pb{g}")
               for g in range(G)]
        # Scalar-queue order: E0 exp, fwd-init i=0, E1 exp, fwd-init i=1,
        # then the (less urgent) backward inits.
        for i in range(2):
            for g in range(G):
                nc.scalar.activation(pts[g][:, i, :],
                                     em_f[:, i, 0, g * GB:(g + 1) * GB],
                                     Exp, bias=st_t[i])
        # round 1's multiplies need the first ee pieces — emit those exps
        # before the (later-needed) backward state inits.
        load_chunk(cf, emf, eef, "f0", mode="first_exp", tiles=tf)
        if not same:
            load_chunk(cb, emb, eeb, "b0", descending=True, mode="first_exp",
                       tiles=tb)
        for i in range(2):
            for g in range(G):
                nc.scalar.activation(uts[g][:, i, :],
                                     em_b[:, i, (n_steps - 1) % W,
                                          g * GB:(g + 1) * GB],
                                     Exp, bias=ben[i])
        p = pts
        u = uts
        # Interleave fwd/bwd rest pieces (Scalar executes exps in order, so
        # a fwd-only run would block all bwd exps behind it), and issue their
        # DMAs on different queues so the descriptors generate in parallel.
        nrest = len(_pieces(W, False, True)) - 1
        for k in range(nrest):
            load_chunk(cf, emf, eef, "f0", mode="rest", tiles=tf,
                       piece_idx=k, dma_eng=nc.gpsimd)
            if not same:
                load_chunk(cb, emb, eeb, "b0", descending=True, mode="rest",
                           tiles=tb, piece_idx=k, dma_eng=nc.sync)

        # GPSIMD/Pool cannot access PSUM on TRN2, so both per-round
        # multiplies stay on DVE (the only PSUM-capable elementwise engine).
        mul_eng = [nc.vector, nc.vector]

        def chain_round(g, state, Emat, qtag, ee_t, w, nm):
            """One MM+mult round for one chain; returns new state."""
            q0 = qpool.tile([128, GB], f32, tag=f"{qtag}0", name=f"{qtag}0")
            q1 = qpool.tile([128, GB], f32, tag=f"{qtag}1", name=f"{qtag}1")
            for j, qj in enumerate((q0, q1)):
                for i in range(2):
                    nc.tensor.matmul(qj, Emat[i][j], state[:, i, :],
                                     start=(i == 0), stop=(i == 1))
            newt = ppool.tile([128, 2, GB], bf16, tag=nm, name=nm)
            for j, qj in enumerate((q0, q1)):
                eesl = ee_t[:, j, w, g * GB:(g + 1) * GB]
                mul_eng[j].tensor_mul(newt[:, j, :], qj, eesl)
            return newt

        # chunk bookkeeping: prefetch the next chunk half-way through the
        # current one (pools are triple-buffered), switch refs at boundaries
        fwd_tiles = {cf: (em_f, ee_f)}
        bwd_tiles = {cb: (em_b, ee_b)}
        cf_hi, cb_lo = cf, cb
        n_rounds = Rf
        for r in range(1, n_rounds + 1):
            sf = r                     # forward step index (uses ee_sf)
            sb = n_steps - 1 - r       # backward: produces u_sb using ee_sb
            if sf <= Rf:
                ahead = min((sf + W // 2) // W, Rf // W)
                if ahead > cf_hi:
                    cf_hi = ahead
                    fwd_tiles[ahead] = load_chunk(ahead, emf, eef, f"f{ahead}")
                em_f, ee_f = fwd_tiles[sf // W]
            if sb >= Rf + 1:
                behind = max((sb - W // 2) // W, (Rf + 1) // W)
                if behind < cb_lo:
                    cb_lo = behind
                    bwd_tiles[behind] = load_chunk(behind, emb, eeb,
                                                   f"b{behind}",
                                                   descending=True,
                                                   dma_eng=nc.sync)
                em_b, ee_b = bwd_tiles[sb // W]
            for g in range(G):
                if sf <= Rf:
                    p[g] = chain_round(g, p[g], E, f"qf{g}", ee_f, sf % W,
                                       f"pf{g}")
                if sb >= Rf + 1:
                    u[g] = chain_round(g, u[g], Et, f"qb{g}", ee_b, sb % W,
                                       f"pb{g}")

        # ---- final: Z = (E^T alpha_Rf)^T u_{Rf+1} ----
        for g in range(G):
            q0 = qpool.tile([128, GB], f32, tag=f"qf{g}0", name=f"qfin{g}0")
            q1 = qpool.tile([128, GB], f32, tag=f"qf{g}1", name=f"qfin{g}1")
            for j, qj in enumerate((q0, q1)):
                for i in range(2):
                    nc.tensor.matmul(qj, E[i][j], p[g][:, i, :],
                                     start=(i == 0), stop=(i == 1))
            d = rpool.tile([128, 2, GB], f32, tag=f"d{g}", name=f"d{g}")
            nc.vector.tensor_mul(d[:, 0, :], q0, u[g][:, 0, :])
            nc.vector.tensor_mul(d[:, 1, :], q1, u[g][:, 1, :])
            fin = spool.tile([1, GB], f32, tag="fin", name=f"fin{g}")
            for i in range(2):
                nc.tensor.matmul(fin, onesf, d[:, i, :],
                                 start=(i == 0), stop=(i == 1))
            res = rpool.tile([1, GB], f32, tag=f"res{g}", name=f"res{g}")
            nc.scalar.copy(res, fin)
            nc.sync.dma_start(out=out[0:1, g * GB:(g + 1) * GB], in_=res)

    if KEEP_MM_WAITS:
        nc.move_matmul_waits_to_ldweights = lambda: None
    nc.compile()
    return nc


def _prep_inputs(emissions, transitions, start_transitions, end_transitions,
                 n_steps=S):
    """Host-side layout prep: per-core input maps."""
    import ml_dtypes
    emissions = np.ascontiguousarray(emissions[:, :n_steps, :], dtype=np.float32)
    em_t = np.ascontiguousarray(emissions.transpose(2, 1, 0)).astype(
        ml_dtypes.bfloat16).reshape(2, 128, n_steps, B)  # [i, p, s, b]
    trm = np.asarray(transitions, np.float32)
    tr = trm.reshape(2, 128, 2, 128)
    trt = np.ascontiguousarray(trm.T).reshape(2, 128, 2, 128)
    import ml_dtypes as _md
    trx = np.ascontiguousarray(np.exp(
        np.concatenate([tr, trt], axis=2),
        dtype=np.float64)).astype(_md.bfloat16)  # [2,128,4,128] = exp(trans)
    st2 = np.asarray(start_transitions, np.float32).reshape(2, 128).T
    en2 = np.asarray(end_transitions, np.float32).reshape(2, 128).T
    sew = np.ascontiguousarray(np.stack([st2, en2], axis=2))  # [128,2,2]
    in_maps = []
    for c in range(NCORES):
        in_maps.append({
            "em": np.ascontiguousarray(em_t[:, :, :, c * BPC:(c + 1) * BPC]),
            "trx": trx, "sew": sew,
        })
    return in_maps


def _gold_score_host(emissions, tags, mask, transitions, start_transitions,
                     end_transitions):
    emissions = np.asarray(emissions, np.float32)
    tags = np.asarray(tags, np.int64)
    m = np.asarray(mask, np.float32)
    emit = np.take_along_axis(emissions, tags[..., None], axis=2)[..., 0]
    trans = np.asarray(transitions, np.float32)[tags[:, :-1], tags[:, 1:]]
    score = (np.asarray(start_transitions, np.float32)[tags[:, 0]] + emit[:, 0]
             + ((emit[:, 1:] + trans) * m[:, 1:]).sum(axis=1))
    last_idx = np.asarray(mask, np.int64).sum(axis=1) - 1
    last_tags = np.take_along_axis(tags, last_idx[:, None], axis=1)[:, 0]
    return score + np.asarray(end_transitions, np.float32)[last_tags]


def _numpy_fallback(emissions, tags, mask, transitions, start_transitions,
                    end_transitions):
    """Reference-faithful numpy path (only used if mask is not all ones)."""
    em = np.asarray(emissions, np.float64)
    msk = np.asarray(mask, bool)
    trn = np.asarray(transitions, np.float64)
    alpha = np.asarray(start_transitions, np.float64)[None, :] + em[:, 0]
    for s in range(1, em.shape[1]):
        scores = alpha[:, :, None] + trn[None, :, :] + em[:, s][:, None, :]
        mx = scores.max(axis=1, keepdims=True)
        new = np.log(np.exp(scores - mx).sum(axis=1)) + mx[:, 0, :]
        alpha = np.where(msk[:, s][:, None], new, alpha)
    fin = alpha + np.asarray(end_transitions, np.float64)[None, :]
    mx = fin.max(axis=1, keepdims=True)
    logden = np.log(np.exp(fin - mx).sum(axis=1)) + mx[:, 0]
    gold = _gold_score_host(emissions, tags, mask, transitions,
                            start_transitions, end_transitions)
    return np.array(np.mean(gold - logden), dtype=np.float32)


def run_device(emissions, transitions, start_transitions, end_transitions,
               n_steps=S, trace=False, tmpdir=None):
    """Compile (cached) + run the Bass kernel; returns (logden[B], results_obj)."""
    from concourse.bass_utils import run_bass_kernel_spmd
    key = n_steps
    if key not in _cache:
        _cache[key] = build_nc(n_steps)
    nc = _cache[key]
    in_maps = _prep_inputs(emissions, transitions, start_transitions,
                           end_transitions, n_steps)
    core_ids = list(range(NCORES))
    r = run_bass_kernel_spmd(nc, in_maps, core_ids, trace=trace, tmpdir=tmpdir)
    zprod = np.concatenate([np.asarray(r.results[c]["out"][0], np.float32)
                            for c in range(NCORES)])
    logden = np.log(zprod) + np.float32((n_steps - 1) * DELTA)
    return logden, r


def kernel(emissions, tags, mask, transitions, start_transitions,
           end_transitions):
    emissions = np.asarray(emissions)
    tags = np.asarray(tags)
    mask = np.asarray(mask)
    if not mask.all():
        return _numpy_fallback(emissions, tags, mask, transitions,
                               start_transitions, end_transitions)
    logden, _ = run_device(emissions, transitions, start_transitions,
                           end_transitions)
    gold = _gold_score_host(emissions, tags, mask, transitions,
                            start_transitions, end_transitions)
    return np.array(np.mean(gold - logden), dtype=np.float32)


# revision 30
# speedup vs baseline: 1.1945x; 1.1945x over previous
"""Trainium2 Bass kernel for CRF mean log-likelihood (B=128, S=512, T=256).

Strategy: data-parallel over batch (16 sequences per core, 8 cores). The
forward-algorithm log-partition is computed in exponential space so the
per-step T x T logsumexp becomes a PE matmul:

    alpha_s = (E^T alpha_{s-1}) * exp(emit_s - delta)     E = exp(trans)

with a constant per-step shift delta ~= log(T) + 1/2 (keeps the state in a
narrow dynamic range; validated drift < +-6 in log space).

The chain is latency-bound (matmul -> elementwise multiply -> matmul), so
the sequence is processed FROM BOTH ENDS simultaneously (meet in the
middle):
  forward:  alpha_s = (E^T alpha_{s-1}) . ee_s          s = 1..Rf
  backward: u_s = (E u_{s+1}) . ee_s   (u_s=gamma_s.ee_s), s = S-2..Rf+1
  Z        = (E^T alpha_Rf)^T  u_{Rf+1}

Per-round critical cycle = mm(172ns fixed SBUF latency) + sem + mult
(175ns, PSUM access) + sem + second-mult serialization.  To cut the last
term the two block-halves' multiplies run on DIFFERENT engines: block 0 on
DVE (vector), block 1 on Pool (gpsimd).  All emission-chunk DMAs issue
from Sync so Pool stays clean; init DMAs are spread across engines and
combined (tr+trt one param, st+en one param) to shorten the pre-chain
head.

The gold (numerator) score is O(B*S) gather work — computed on host.
"""
import numpy as np

B, S, T = 128, 512, 256
NCORES = 8
BPC = B // NCORES          # batch per core = 16
G = 1                      # batch groups per core (chains = 2*G)
GB = BPC // G
W = 128                    # steps per emissions chunk
FIRST = 8                  # steps in the init-critical first piece
BULK = 32                  # steps per bulk DMA/exp piece
DELTA = 6.045              # per-step log-space shift ~ log(256) + 0.5
KEEP_MM_WAITS = True       # skip bacc's move_matmul_waits_to_ldweights

_cache = {}


def _pieces(n, descending, first_small):
    """Piece boundaries for chunk streaming, in consumption order.  First
    chunks ramp FIRST -> BULK -> 2*BULK; steady-state chunks use 2*BULK
    pieces (fewer piece boundaries = fewer DVE sem-guard bubbles)."""
    widths, rem = [], n
    for w in ([FIRST, BULK] if first_small else []):
        w = min(w, rem)
        if w:
            widths.append(w)
            rem -= w
    while rem > 0:
        w = min(2 * BULK, rem)
        widths.append(w)
        rem -= w
    pieces, pos = [], 0
    for w in widths:
        pieces.append((pos, pos + w))
        pos += w
    if descending:
        pieces = [(n - b, n - a) for a, b in pieces]
    return pieces


def build_nc(n_steps=S):
    import concourse.bass as bass
    import concourse.tile as tile
    from concourse import bacc, mybir
    from contextlib import ExitStack

    f32 = mybir.dt.float32
    bf16 = mybir.dt.bfloat16
    Exp = mybir.ActivationFunctionType.Exp

    assert n_steps >= 4
    Rf = (n_steps - 2) // 2          # forward DVE-rounds (alpha_1..alpha_Rf)

    nc = bacc.Bacc()
    em = nc.declare_dram_parameter("em", [2, 128, n_steps, BPC], bf16,
                                   isOutput=False)
    # trx[i] = [tr[i] blocks j=0,1 | trt[i] blocks j=0,1] along slot dim
    trx = nc.declare_dram_parameter("trx", [2, 128, 4, 128], bf16,
                                    isOutput=False)
    # sew[p, i, c]: c=0 start, c=1 end; i = contraction half
    sew = nc.declare_dram_parameter("sew", [128, 2, 2], f32, isOutput=False)
    out = nc.declare_dram_parameter("out", [1, BPC], f32, isOutput=True)

    with ExitStack() as ctx:
        tc = ctx.enter_context(tile.TileContext(nc))
        const = ctx.enter_context(tc.tile_pool(name="const", bufs=1))
        emf = ctx.enter_context(tc.tile_pool(name="emf", bufs=3))
        eef = ctx.enter_context(tc.tile_pool(name="eef", bufs=3))
        emb = ctx.enter_context(tc.tile_pool(name="emb", bufs=3))
        eeb = ctx.enter_context(tc.tile_pool(name="eeb", bufs=3))
        ppool = ctx.enter_context(tc.tile_pool(name="p", bufs=4))
        rpool = ctx.enter_context(tc.tile_pool(name="rn", bufs=2))
        qpool = ctx.enter_context(tc.tile_pool(name="q", bufs=1, space="PSUM"))
        spool = ctx.enter_context(tc.tile_pool(name="s", bufs=2, space="PSUM"))

        # ---- one-time constants ----
        # Weight staging: one DMA + one big exp ACT per contraction half i.
        # Eall[i][:, 0:2, :] = E blocks (fwd lhsT), [:, 2:4, :] = Et (bwd).
        # trx already holds exp(transitions) (computed on host), so the
        # weight tiles are DMA'd directly — no device-side exp ACTs on the
        # first-matmul critical path.
        Eall = [const.tile([128, 4, 128], bf16, tag=f"Eall{i}", name=f"Eall{i}")
                for i in range(2)]
        for i in range(2):
            nc.sync.dma_start(out=Eall[i], in_=trx[i])
        E = [[Eall[i][:, j, :] for j in range(2)] for i in range(2)]
        Et = [[Eall[i][:, 2 + j, :] for j in range(2)] for i in range(2)]

        sews = rpool.tile([128, 2, 2], f32, tag="sewstage", name="sewstage")
        nc.gpsimd.dma_start(out=sews, in_=sew[:, :, :])
        onesf = const.tile([128, 1], f32, tag="onesf", name="onesf")
        nc.vector.memset(onesf, 1.0)
        dbias = const.tile([128, 1], f32, tag="dbias", name="dbias")
        nc.vector.memset(dbias, -DELTA)
        st_t = [sews[:, i, 0:1] for i in range(2)]
        ben = []
        for i in range(2):
            t = const.tile([128, 1], f32, tag=f"ben{i}", name=f"ben{i}")
            nc.vector.tensor_add(t, sews[:, i, 1:2], dbias)  # end - delta
            ben.append(t)

        # ---- emissions chunk streaming (per direction) ----
        # Stream each chunk in pieces (DMA pair + exp ACT per piece), ordered
        # by consumption direction; all chunk DMAs issue from Sync so the
        # Pool engine stays free for the per-round multiplies.
        def load_chunk(c, pool, eepool_, nm, descending=False,
                       mode="full", tiles=None, dma_eng=None, piece_idx=None):
            """mode: full (fresh chunk, dma+exp) | first_dma (alloc, dma of
            the small first piece only) | first_exp (exp of that piece) |
            rest (dma+exp of remaining pieces; piece_idx selects one)."""
            s0, s1 = c * W, min(n_steps, (c + 1) * W)
            n = s1 - s0
            if tiles is None:
                t = pool.tile([128, 2, W, BPC], bf16, tag="emchunk",
                              name=f"em{nm}")
                te = eepool_.tile([128, 2, W, BPC], bf16, tag="eechunk",
                                  name=f"ee{nm}")
            else:
                t, te = tiles
            pieces = _pieces(n, descending, mode != "full")
            if mode in ("first_dma", "first_exp"):
                pieces = pieces[:1]
            elif mode == "rest":
                pieces = pieces[1:]
                if piece_idx is not None:
                    pieces = pieces[piece_idx:piece_idx + 1]
            for a, b in pieces:
                if mode != "first_exp":
                    for i in range(2):
                        (dma_eng or nc.gpsimd).dma_start(
                            out=t[:, i, a:b, :],
                            in_=em[i, :, s0 + a:s0 + b, :])
                if mode != "first_dma":
                    nc.scalar.activation(te[:, :, a:b, :], t[:, :, a:b, :],
                                         Exp, bias=dbias)
            return t, te

        # ---- chain state ----
        # First the two init-critical pieces + the state inits, then the bulk
        # of both chunks — keeps the first matmul off the DMA/ACT queues.
        cf = 0                       # forward chunk index
        cb = (n_steps - 1) // W      # backward chunk index
        tf = load_chunk(cf, emf, eef, "f0", mode="first_dma",
                        dma_eng=nc.gpsimd)
        same = (cb == cf)
        tb = tf if same else load_chunk(cb, emb, eeb, "b0", descending=True,
                                        mode="first_dma", dma_eng=nc.sync)
        em_f, ee_f = tf
        em_b, ee_b = tb

        p = []   # forward states per group
        u = []   # backward states per group
        pts = [ppool.tile([128, 2, GB], bf16, tag=f"pf{g}", name=f"pf{g}")
               for g in range(G)]
        uts = [ppool.tile([128, 2, GB], bf16, tag=f"pb{g}", name=f"pb{g}")
               for g in range(G)]
        # Scalar-queue order: E0 exp, fwd-init i=0, E1 exp, fwd-init i=1,
        # then the (less urgent) backward inits.
        for i in range(2):
            for g in range(G):
                nc.scalar.activation(pts[g][:, i, :],
                                     em_f[:, i, 0, g * GB:(g + 1) * GB],
                                     Exp, bias=st_t[i])
        # round 1's multiplies need the first ee pieces — emit those exps
        # before the (later-needed) backward state inits.
        load_chunk(cf, emf, eef, "f0", mode="first_exp", tiles=tf)
        if not same:
            load_chunk(cb, emb, eeb, "b0", descending=True, mode="first_exp",
                       tiles=tb)
        for i in range(2):
            for g in range(G):
                nc.scalar.activation(uts[g][:, i, :],
                                     em_b[:, i, (n_steps - 1) % W,
                                          g * GB:(g + 1) * GB],
                                     Exp, bias=ben[i])
        p = pts
        u = uts
        # Interleave fwd/bwd rest pieces (Scalar executes exps in order, so
        # a fwd-only run would block all bwd exps behind it), and issue their
        # DMAs on different queues so the descriptors generate in parallel.
        nrest = len(_pieces(W, False, True)) - 1
        for k in range(nrest):
            load_chunk(cf, emf, eef, "f0", mode="rest", tiles=tf,
                       piece_idx=k, dma_eng=nc.gpsimd)
            if not same:
                load_chunk(cb, emb, eeb, "b0", descending=True, mode="rest",
                           tiles=tb, piece_idx=k, dma_eng=nc.sync)

        # GPSIMD/Pool cannot access PSUM on TRN2, so both per-round
        # multiplies stay on DVE (the only PSUM-capable elementwise engine).
        mul_eng = [nc.vector, nc.vector]

        def chain_round(g, state, Emat, qtag, ee_t, w, nm):
            """One MM+mult round for one chain; returns new state."""
            q0 = qpool.tile([128, GB], f32, tag=f"{qtag}0", name=f"{qtag}0")
            q1 = qpool.tile([128, GB], f32, tag=f"{qtag}1", name=f"{qtag}1")
            for j, qj in enumerate((q0, q1)):
                for i in range(2):
                    nc.tensor.matmul(qj, Emat[i][j], state[:, i, :],
                                     start=(i == 0), stop=(i == 1))
            newt = ppool.tile([128, 2, GB], bf16, tag=nm, name=nm)
            for j, qj in enumerate((q0, q1)):
                eesl = ee_t[:, j, w, g * GB:(g + 1) * GB]
                mul_eng[j].tensor_mul(newt[:, j, :], qj, eesl)
            return newt

        # chunk bookkeeping: prefetch the next chunk half-way through the
        # current one (pools are triple-buffered), switch refs at boundaries
        fwd_tiles = {cf: (em_f, ee_f)}
        bwd_tiles = {cb: (em_b, ee_b)}
        cf_hi, cb_lo = cf, cb
        n_rounds = Rf
        for r in range(1, n_rounds + 1):
            sf = r                     # forward step index (uses ee_sf)
            sb = n_steps - 1 - r       # backward: produces u_sb using ee_sb
            if sf <= Rf:
                ahead = min((sf + W // 2) // W, Rf // W)
                if ahead > cf_hi:
                    cf_hi = ahead
                    fwd_tiles[ahead] = load_chunk(ahead, emf, eef, f"f{ahead}")
                em_f, ee_f = fwd_tiles[sf // W]
            if sb >= Rf + 1:
                behind = max((sb - W // 2) // W, (Rf + 1) // W)
                if behind < cb_lo:
                    cb_lo = behind
                    bwd_tiles[behind] = load_chunk(behind, emb, eeb,
                                                   f"b{behind}",
                                                   descending=True,
                                                   dma_eng=nc.sync)
                em_b, ee_b = bwd_tiles[sb // W]
            for g in range(G):
                if sf <= Rf:
                    p[g] = chain_round(g, p[g], E, f"qf{g}", ee_f, sf % W,
                                       f"pf{g}")
                if sb >= Rf + 1:
                    u[g] = chain_round(g, u[g], Et, f"qb{g}", ee_b, sb % W,
                                       f"pb{g}")

        # ---- final: Z = (E^T alpha_Rf)^T u_{Rf+1} ----
        for g in range(G):
            q0 = qpool.tile([128, GB], f32, tag=f"qf{g}0", name=f"qfin{g}0")
            q1 = qpool.tile([128, GB], f32, tag=f"qf{g}1", name=f"qfin{g}1")
            for j, qj in enumerate((q0, q1)):
                for i in range(2):
                    nc.tensor.matmul(qj, E[i][j], p[g][:, i, :],
                                     start=(i == 0), stop=(i == 1))
            d = rpool.tile([128, 2, GB], f32, tag=f"d{g}", name=f"d{g}")
            nc.vector.tensor_mul(d[:, 0, :], q0, u[g][:, 0, :])
            nc.vector.tensor_mul(d[:, 1, :], q1, u[g][:, 1, :])
            fin = spool.tile([1, GB], f32, tag="fin", name=f"fin{g}")
            for i in range(2):
                nc.tensor.matmul(fin, onesf, d[:, i, :],
                                 start=(i == 0), stop=(i == 1))
            res = rpool.tile([1, GB], f32, tag=f"res{g}", name=f"res{g}")
            nc.scalar.copy(res, fin)
            nc.sync.dma_start(out=out[0:1, g * GB:(g + 1) * GB], in_=res)

    if KEEP_MM_WAITS:
        nc.move_matmul_waits_to_ldweights = lambda: None
    nc.compile()
    return nc


def _prep_inputs(emissions, transitions, start_transitions, end_transitions,
                 n_steps=S):
    """Host-side layout prep: per-core input maps."""
    import ml_dtypes
    emissions = np.ascontiguousarray(emissions[:, :n_steps, :], dtype=np.float32)
    em_t = np.ascontiguousarray(emissions.transpose(2, 1, 0)).astype(
        ml_dtypes.bfloat16).reshape(2, 128, n_steps, B)  # [i, p, s, b]
    trm = np.asarray(transitions, np.float32)
    tr = trm.reshape(2, 128, 2, 128)
    trt = np.ascontiguousarray(trm.T).reshape(2, 128, 2, 128)
    import ml_dtypes as _md
    trx = np.ascontiguousarray(np.exp(
        np.concatenate([tr, trt], axis=2),
        dtype=np.float64)).astype(_md.bfloat16)  # [2,128,4,128] = exp(trans)
    st2 = np.asarray(start_transitions, np.float32).reshape(2, 128).T
    en2 = np.asarray(end_transitions, np.float32).reshape(2, 128).T
    sew = np.ascontiguousarray(np.stack([st2, en2], axis=2))  # [128,2,2]
    in_maps = []
    for c in range(NCORES):
        in_maps.append({
            "em": np.ascontiguousarray(em_t[:, :, :, c * BPC:(c + 1) * BPC]),
            "trx": trx, "sew": sew,
        })
    return in_maps


def _gold_score_host(emissions, tags, mask, transitions, start_transitions,
                     end_transitions):
    emissions = np.asarray(emissions, np.float32)
    tags = np.asarray(tags, np.int64)
    m = np.asarray(mask, np.float32)
    emit = np.take_along_axis(emissions, tags[..., None], axis=2)[..., 0]
    trans = np.asarray(transitions, np.float32)[tags[:, :-1], tags[:, 1:]]
    score = (np.asarray(start_transitions, np.float32)[tags[:, 0]] + emit[:, 0]
             + ((emit[:, 1:] + trans) * m[:, 1:]).sum(axis=1))
    last_idx = np.asarray(mask, np.int64).sum(axis=1) - 1
    last_tags = np.take_along_axis(tags, last_idx[:, None], axis=1)[:, 0]
    return score + np.asarray(end_transitions, np.float32)[last_tags]


def _numpy_fallback(emissions, tags, mask, transitions, start_transitions,
                    end_transitions):
    """Reference-faithful numpy path (only used if mask is not all ones)."""
    em = np.asarray(emissions, np.float64)
    msk = np.asarray(mask, bool)
    trn = np.asarray(transitions, np.float64)
    alpha = np.asarray(start_transitions, np.float64)[None, :] + em[:, 0]
    for s in range(1, em.shape[1]):
        scores = alpha[:, :, None] + trn[None, :, :] + em[:, s][:, None, :]
        mx = scores.max(axis=1, keepdims=True)
        new = np.log(np.exp(scores - mx).sum(axis=1)) + mx[:, 0, :]
        alpha = np.where(msk[:, s][:, None], new, alpha)
    fin = alpha + np.asarray(end_transitions, np.float64)[None, :]
    mx = fin.max(axis=1, keepdims=True)
    logden = np.log(np.exp(fin - mx).sum(axis=1)) + mx[:, 0]
    gold = _gold_score_host(emissions, tags, mask, transitions,
                            start_transitions, end_transitions)
    return np.array(np.mean(gold - logden), dtype=np.float32)


def run_device(emissions, transitions, start_transitions, end_transitions,
               n_steps=S, trace=False, tmpdir=None):
    """Compile (cached) + run the Bass kernel; returns (logden[B], results_obj)."""
    from concourse.bass_utils import run_bass_kernel_spmd
    key = n_steps
    if key not in _cache:
        _cache[key] = build_nc(n_steps)
    nc = _cache[key]
    in_maps = _prep_inputs(emissions, transitions, start_transitions,
                           end_transitions, n_steps)
    core_ids = list(range(NCORES))
    r = run_bass_kernel_spmd(nc, in_maps, core_ids, trace=trace, tmpdir=tmpdir)
    zprod = np.concatenate([np.asarray(r.results[c]["out"][0], np.float32)
                            for c in range(NCORES)])
    logden = np.log(zprod) + np.float32((n_steps - 1) * DELTA)
    return logden, r


def kernel(emissions, tags, mask, transitions, start_transitions,
           end_transitions):
    emissions = np.asarray(emissions)
    tags = np.asarray(tags)
    mask = np.asarray(mask)
    if not mask.all():
        return _numpy_fallback(emissions, tags, mask, transitions,
                               start_transitions, end_transitions)
    logden, _ = run_device(emissions, transitions, start_transitions,
                           end_transitions)
    gold = _gold_score_host(emissions, tags, mask, transitions,
                            start_transitions, end_transitions)
    return np.array(np.mean(gold - logden), dtype=np.float32)
